# revision 1
# baseline (speedup 1.0000x reference)
"""Banded multi-head attention (B=2, L=1024, D=1024, H=16, band W=64) on 8
Trainium2 NeuronCores.

Sharding: core = (batch b, head-group g), 2 batches x 4 head groups of 4
heads.  Each core computes q/k/v projections for its group, the banded
attention for its 4 heads, and a partial output projection through its
slice of Wo.  Host sums the 4 bf16 partial outputs per batch in f32.

v2 redesign vs the f32r baseline (46.7us):
- Everything bf16 (rel err ~2.3e-3 vs 2e-2 budget).  bf16 matmuls run at
  1 row/cycle at ANY moving size (f32r needs >=256), which unlocks small
  64-query attention tiles; DMA bytes halve.
- Attention at 64-query tiles: the 128-key window [t0-64, t0+64) covers
  the whole causal band of a 64-query tile, so scores are ONE stationary
  load per (head, tile): kT window is free-dim sliced (any offset), and V
  is kept at TWO partition alignments (v_sb token-aligned, v2_sb shifted
  by 64 via one sbuf->sbuf DMA) so attn@V is also a single 128-key chunk.
  Score rows drop 3x (12288 -> 4096+4096 rows + no 3-chunk masks).
- Softmax denominators via gpsimd partition_all_reduce over the masked
  e tiles (sum over keys broadcast to all partitions) + one deferred DVE
  reciprocal per block -- runs before/during attn@V so the o-normalize
  multiply is never chain-blocked.  No ones-column, no broadcast matmul.
- 1/sqrt(dh) folded into Wq on host.  Band mask is a single resident
  [128, 512] 0/1 bf16 tile applied multiplicatively after exp (Pool).
- DMA triggers collapsed ~97 -> ~25, each one contiguous run per
  partition (host pre-rearranges to [128, X]; DGE trigger cost scales
  with descriptor count); consts DMA'd outside the loop; x loads issue
  from the gpsimd queue, y stores alternate SP/gpsimd.
- Emission order hand-pipelined: scores run 1-2 blocks ahead of attn@V;
  second-half projections (split at the tok-704 band boundary so block-3
  scores start early) and output projections are the PE fillers that
  hide exp/mask and denominator-chain latencies.
- The timing loop unrolls up to 16 kernel evaluations per For_i trip
  (largest of 16/8/4/2 dividing loop_n; deeper unrolls simulated faster
  but their NEFF compile time is unvalidated on hardware) with
  double-buffered x, explicit cross-body prefetch (each body issues the
  NEXT body's x load mid-stream) and cross-body PE fillers (each body's
  tail emits the next body's first two projection chains into the
  denominator-chain wait gaps, with a pre-loop primer pair so all
  bodies are uniform).  Every body starts with x resident and zero
  head-chain latency; its store tail drains under successor compute.
  The single-shot path (kernel()) emits one plain body.
PE work: 65536 projection rows + 8192 attention rows = 30.7us floor
(proven minimal: with dh=64, one moving row yields <= 64x128 useful
score MACs, so 1 row/query/head is the bound and this kernel is on it);
CoreSim: cold 39.8us (vs 97.4us baseline), PE busy 31.2us (vs 45.9us),
looped steady-state 31.9us/iter at unroll-16 (2.2% over the PE floor).
"""
import numpy as np
import ml_dtypes

import concourse.bacc as bacc
import concourse.bass_isa as bass_isa
import concourse.mybir as mybir
import concourse.tile as tile
from concourse import bass_utils

B, L, D, H, W = 2, 1024, 1024, 16, 64
DH = D // H           # 64
G = 4                 # head groups
HPG = H // G          # 4 heads per group
DG = D // G           # 256 dims per group
NCORES = 8

VSLOT = DH            # 64 cols per (slot, head)
VROW = HPG * VSLOT    # 256 cols per key slot
NSLOT = 8             # 128-key slots

F32 = mybir.dt.float32
BF16 = mybir.dt.bfloat16
EXPF = mybir.ActivationFunctionType.Exp
MULT = None  # set lazily


def _pin_exp_table(arch: str):
    """Resolve Copy/Exp only to the natural_log_exp_and_others act-func
    set so exactly one table load is emitted (alternating per-function
    table swaps wedge the device)."""
    import concourse.hw_specs as hw_specs
    tables = hw_specs.get_activation_tables(arch)   # cached, mutable
    drop = {EXPF, mybir.ActivationFunctionType.Ln,
            mybir.ActivationFunctionType.Copy,
            mybir.ActivationFunctionType.Identity}
    assert "natural_log_exp_and_others" in tables
    for name, funcs in tables.items():
        if name != "natural_log_exp_and_others":
            funcs -= drop


def build(loop_n: int = 0):
    """Build + compile the per-core Bass program.  loop_n > 0 wraps the
    body in a device-side For_i executing it loop_n times (HW timing)."""
    from concourse.alu_op_type import AluOpType
    nc = bacc.Bacc("TRN2", target_bir_lowering=False, debug=False)
    _pin_exp_table(nc.m.arch)

    # Host pre-rearranges big tensors to [128, ...] so every DMA is one
    # contiguous run per partition (descriptor-count == 128; the DGE
    # trigger cost scales with descriptor count).
    xT = nc.dram_tensor("xT", [128, 2 * 8 * 512], BF16, kind="ExternalInput")
    wqkv = nc.dram_tensor("wqkv", [128, 8 * 768], BF16, kind="ExternalInput")
    woT = nc.dram_tensor("woT", [128, 2 * D], BF16, kind="ExternalInput")
    kcd = nc.dram_tensor("kc", [64, HPG * 64], BF16, kind="ExternalInput")
    vcd = nc.dram_tensor("vc", [64, VROW], BF16, kind="ExternalInput")
    maskd = nc.dram_tensor("mask", [128, 512], BF16, kind="ExternalInput")
    y = nc.dram_tensor("y", [L, D], BF16, kind="ExternalOutput")

    with tile.TileContext(nc) as tc:
        with tc.tile_pool(name="res", bufs=1) as res, \
             tc.tile_pool(name="epool", bufs=4) as epool, \
             tc.tile_pool(name="rcpool", bufs=4) as rcpool, \
             tc.tile_pool(name="bcpool", bufs=4) as bcpool, \
             tc.tile_pool(name="ypool", bufs=3) as ypool, \
             tc.tile_pool(name="ps", bufs=8, space="PSUM") as psp:

            # ---- resident SBUF tensors ----------------------------------
            # xk col layout: n*4096 + k*512 + c  (n = token half, k = K
            # chunk, c = token within half) — matches the xT host layout.
            xkb = [res.tile([128, 8 * L], BF16, tag=f"xk{r}",
                             name=f"xk{r}") for r in range(2)]
            xkc = [xkb[0]]   # current rep's x buffer
            wsb = res.tile([128, 8 * 3 * DG], BF16, tag="w", name="w_sb")
            wo_sb = res.tile([128, 2 * D], BF16, tag="wo", name="wo_sb")
            qT = res.tile([64, HPG * L], BF16, tag="qT", name="qT")
            kT = res.tile([64, HPG * (64 + L)], BF16, tag="kT", name="kT")
            v_sb = res.tile([128, NSLOT * VROW], BF16, tag="v", name="v_sb")
            # v2 split into slot-pair tiles so attnV blocks only depend on
            # their own shift DMA (dependency tracking is per-tile)
            v2p = [res.tile([128, 2 * VROW], BF16, tag=f"v2_{i}",
                            name=f"v2_{i}") for i in range(4)]
            mask_sb = res.tile([128, 512], BF16, tag="mask", name="mask_sb")
            oT = [res.tile([128, L], BF16, tag=f"oT{m}", name=f"oT{m}")
                  for m in range(2)]

            KW = 64 + L  # kT cols per head

            def emit_qkproj(proj, m, n, c0=0, cw=512):
                """proj 0 = Q -> qT, 1 = K -> kT; m = head pair, n = token
                half, [c0, c0+cw) column window within the half.  psum
                [128 dims, cw tok] accumulated over 8 K-chunks, then
                per-head [64, cw] copies (Act + DVE)."""
                pt = psp.tile([128, 512], F32, tag="ps", name="pj")
                for k in range(8):
                    nc.tensor.matmul(
                        pt[:, 0:cw],
                        wsb[:, k * 512 + proj * 256 + m * 128:
                            k * 512 + proj * 256 + m * 128 + 128],
                        xkc[0][:, n * 4096 + k * 512 + c0:
                               n * 4096 + k * 512 + c0 + cw],
                        start=(k == 0), stop=(k == 7),
                    )
                for hh in range(2):
                    h = 2 * m + hh
                    if proj == 0:
                        dst = qT[:, h * L + n * 512 + c0:
                                 h * L + n * 512 + c0 + cw]
                    else:
                        dst = kT[:, h * KW + 64 + n * 512 + c0:
                                 h * KW + 64 + n * 512 + c0 + cw]
                    src = pt[hh * 64:(hh + 1) * 64, 0:cw]
                    if hh == 0:
                        nc.scalar.copy(dst, src)
                    else:
                        nc.vector.tensor_copy(dst, src)

            def emit_vproj(s):
                """Token slot s (128 tokens) -> v_sb B-slot s."""
                pv = psp.tile([128, 512], F32, tag="ps", name="pjv")
                n, c0 = divmod(s, 4)
                for k in range(8):
                    base = n * 4096 + k * 512 + c0 * 128
                    nc.tensor.matmul(
                        pv[:, 0:DG],
                        xkc[0][:, base: base + 128],
                        wsb[:, 4096 + k * 256: 4096 + k * 256 + 256],
                        start=(k == 0), stop=(k == 7),
                    )
                nc.vector.tensor_copy(v_sb[:, s * VROW:(s + 1) * VROW],
                                      pv[:, 0:DG])

            eblk = {}

            def emit_attn_block(bi, phase):
                """256 queries [256*bi, 256*bi+256), all 4 heads.
                Tiles j = 4bi .. 4bi+3 (64 queries each); per 128q group one
                [128, 512] score psum (2 tiles x 4 heads x 64q).  phase
                'scores' emits scores+exp+mask; 'attnv' the rest."""
                if phase == "scores":
                    es = []
                    for gg in range(2):
                        st = psp.tile([128, 512], F32, tag="ps", name="st")
                        for jj in range(2):
                            j = 4 * bi + 2 * gg + jj
                            for h in range(HPG):
                                nc.tensor.matmul(
                                    st[:, jj * 256 + h * 64:
                                       jj * 256 + h * 64 + 64],
                                    kT[:, h * KW + 64 * j:
                                       h * KW + 64 * j + 128],
                                    qT[:, h * L + 64 * j:
                                       h * L + 64 * j + 64],
                                    start=True, stop=True,
                                )
                        e = epool.tile([128, 512], BF16, tag="e", name="e")
                        nc.scalar.activation(e[:], st[:], EXPF)
                        nc.gpsimd.tensor_mul(e[:], e[:], mask_sb[:])
                        es.append(e)
                    # denominators AFTER both masks (so the Pool queue never
                    # delays e readiness): per-group sum over keys broadcast
                    # to all partitions, then one block-wide reciprocal --
                    # all complete before/during attnV, so o-mult is never
                    # chain-blocked
                    rbt = rcpool.tile([128, 1024], F32, tag="rb", name="rb")
                    for gg in range(2):
                        nc.gpsimd.partition_all_reduce(
                            rbt[:, gg * 512:(gg + 1) * 512], es[gg][:],
                            128, bass_isa.ReduceOp.add)
                    eblk[bi] = (es, rbt)
                    return
                if phase == "recip":
                    nc.vector.reciprocal(eblk[bi][1][:], eblk[bi][1][:])
                    return
                es, rbt = eblk.pop(bi)
                for m in range(2):
                    op = psp.tile([64, 512], F32, tag="ps", name="op")
                    for hh in range(2):
                        h = 2 * m + hh
                        for q4 in range(4):
                            j = 4 * bi + q4
                            slot = j // 2
                            if j % 2 == 0:
                                vt, so = v2p[slot // 2], slot % 2
                            else:
                                vt, so = v_sb, slot
                            nc.tensor.matmul(
                                op[0:64, hh * 256 + q4 * 64:
                                   hh * 256 + q4 * 64 + 64],
                                vt[:, so * VROW + h * VSLOT:
                                   so * VROW + h * VSLOT + VSLOT],
                                es[q4 // 2][:, (q4 % 2) * 256 + h * 64:
                                             (q4 % 2) * 256 + h * 64 + 64],
                                start=True, stop=True,
                            )
                    for hh in range(2):
                        h = 2 * m + hh
                        dst = oT[m][hh * 64:(hh + 1) * 64,
                                    256 * bi: 256 * bi + 256]
                        nc.vector.tensor_mul(
                            dst.rearrange("p (g c) -> p g c", c=64),
                            op[0:64, hh * 256: hh * 256 + 256].rearrange(
                                "p (g c) -> p g c", c=64),
                            rbt[0:64, :].rearrange(
                                "p (g x) -> p g x", g=4)
                            [:, :, h * 64: h * 64 + 64])

            def emit_oproj(t):
                """Token tile t (128 tokens) -> y row block (bf16).  Each
                512-col half is DMA'd as soon as its copy lands."""
                ysb = ypool.tile([128, D], BF16, tag="y", name="ysb")
                for n2 in range(2):
                    yp = psp.tile([128, 512], F32, tag="ps", name="yp")
                    for m in range(2):
                        nc.tensor.matmul(
                            yp[:],
                            oT[m][:, t * 128:(t + 1) * 128],
                            wo_sb[:, m * D + n2 * 512: m * D + n2 * 512 + 512],
                            start=(m == 0), stop=(m == 1),
                        )
                    if n2 == 0:
                        nc.scalar.copy(ysb[:, 0:512], yp[:])
                    else:
                        nc.vector.tensor_copy(ysb[:, 512:1024], yp[:])
                    eng = nc.sync if (t + n2) % 2 == 0 else nc.gpsimd
                    eng.dma_start(
                        y.ap()[t * 128:(t + 1) * 128,
                               n2 * 512:(n2 + 1) * 512],
                        ysb[:, n2 * 512:(n2 + 1) * 512])

            def emit_vshift(a):
                """v2 A-slots {a, a+1} (keys [128a-64, 128a+192)) via
                partition-shifted sbuf->sbuf copies from B-slots."""
                vt = v2p[a // 2]
                nc.sync.dma_start(vt[64:128, :],
                                  v_sb[0:64, a * VROW:(a + 2) * VROW])
                lo = max(a, 1)   # A0 lower half is the const cache block
                nc.sync.dma_start(
                    vt[0:64, (lo - a) * VROW:2 * VROW],
                    v_sb[64:128, (lo - 1) * VROW:(a + 1) * VROW])

            # ---- constant inputs: outside the timing loop ----------------
            # Split so the first qk weight chunks land ASAP on the cold run.
            nc.sync.dma_start(wsb[:, 0:512], wqkv.ap()[:, 0:512])
            nc.sync.dma_start(wsb[:, 512:2048], wqkv.ap()[:, 512:2048])
            nc.sync.dma_start(wsb[:, 2048:4096], wqkv.ap()[:, 2048:4096])
            nc.sync.dma_start(wsb[:, 4096:6144], wqkv.ap()[:, 4096:6144])
            nc.sync.dma_start(
                kT[:].rearrange("p (h c) -> p h c", c=KW)[:, :, 0:64],
                kcd.ap().rearrange("p (h c) -> p h c", c=64))
            nc.sync.dma_start(v2p[0][0:64, 0:VROW], vcd.ap())
            nc.sync.dma_start(mask_sb[:], maskd.ap())
            nc.sync.dma_start(wo_sb[:], woT.ap())

            import contextlib

            def rep_ctx(trips):
                if trips:
                    return tc.For_i(0, trips, 1,
                                    hint_engines=(mybir.EngineType.PE,
                                                  mybir.EngineType.Activation,
                                                  mybir.EngineType.DVE,
                                                  mybir.EngineType.Pool,
                                                  mybir.EngineType.SP))
                return contextlib.nullcontext()

            def emit_xload(buf):
                xk = xkb[buf]
                nc.gpsimd.dma_start(xk[:, 0:512], xT.ap()[:, 0:512])
                nc.gpsimd.dma_start(xk[:, 512:2048], xT.ap()[:, 512:2048])
                nc.gpsimd.dma_start(xk[:, 2048:4096], xT.ap()[:, 2048:4096])
                nc.gpsimd.dma_start(xk[:, 4096:8192], xT.ap()[:, 4096:8192])

            def emit_body(rep, prefetch_buf=None, skip_first=False,
                          tail_next=None):
                # Double-buffered x.  prefetch_buf=None: load this body's
                # buffer at its head (cold / non-unrolled path).  Otherwise
                # the buffer was loaded by the previous body (or the
                # pre-loop load) and this body instead prefetches the NEXT
                # body's buffer mid-stream, so the following body starts
                # with x fully resident.
                xkc[0] = xkb[rep]
                if prefetch_buf is None:
                    emit_xload(rep)

                # ---- first half: tok 0-511 ------------------------------
                # the m=0 chains may have been emitted as the previous
                # body's tail fillers (cross-body PE fill)
                if not skip_first:
                    emit_qkproj(0, 0, 0)
                    emit_qkproj(1, 0, 0)
                emit_qkproj(0, 1, 0)
                emit_qkproj(1, 1, 0)
                emit_vproj(0)
                emit_vproj(1)
                emit_vshift(0)
                # scores run ahead of attnV; second-half projection chains
                # are the PE fillers that hide exp/mask and denominator
                # chain latencies
                emit_attn_block(0, phase="scores")
                emit_attn_block(0, phase="recip")
                emit_vproj(2)
                emit_vproj(3)
                emit_vshift(2)
                emit_attn_block(1, phase="scores")
                emit_attn_block(1, phase="recip")
                emit_attn_block(0, phase="attnv")
                emit_qkproj(0, 0, 1, 192, 320)
                emit_attn_block(1, phase="attnv")
                if prefetch_buf is not None:
                    emit_xload(prefetch_buf)
                emit_qkproj(1, 0, 1, 192, 320)
                # ---- second half: tok 512-1023 --------------------------
                # qk-n1 is emitted band-aligned: tok [704, 1024) for all
                # head pairs first, so sc-b3 (queries 768+, keys 704+) can
                # start while tok [512, 704) is still projecting; all
                # oprojs act as PE fillers that hide the attention chains;
                # b3 runs before b2 so the last oprojs wait on the earlier
                # denominator chain
                emit_qkproj(0, 1, 1, 192, 320)
                emit_qkproj(1, 1, 1, 192, 320)
                emit_attn_block(3, phase="scores")
                emit_qkproj(0, 0, 1, 0, 192)
                emit_qkproj(1, 0, 1, 0, 192)
                emit_qkproj(0, 1, 1, 0, 192)
                emit_qkproj(1, 1, 1, 0, 192)
                emit_attn_block(2, phase="scores")
                emit_attn_block(3, phase="recip")
                emit_vproj(5)
                emit_vproj(6)
                emit_vproj(7)
                emit_vshift(6)
                emit_vproj(4)
                emit_vshift(4)
                emit_attn_block(2, phase="recip")
                emit_oproj(0)
                emit_oproj(1)
                emit_attn_block(3, phase="attnv")
                emit_oproj(2)
                emit_attn_block(2, phase="attnv")
                emit_oproj(3)
                if tail_next is not None:
                    xkc[0] = xkb[tail_next]
                    emit_qkproj(0, 0, 0)
                emit_oproj(6)
                emit_oproj(7)
                if tail_next is not None:
                    emit_qkproj(1, 0, 0)
                    xkc[0] = xkb[rep]
                emit_oproj(4)
                emit_oproj(5)

            def unrolled(nb):
                emit_xload(0)
                xkc[0] = xkb[0]
                emit_qkproj(0, 0, 0)   # primer for the first body
                emit_qkproj(1, 0, 0)
                with rep_ctx(loop_n // nb):
                    for r in range(nb):
                        emit_body(r % 2, prefetch_buf=(r + 1) % 2,
                                  skip_first=True,
                                  tail_next=(r + 1) % 2)

            if loop_n == 0:
                emit_body(0)
            elif loop_n % 16 == 0:
                unrolled(16)
            elif loop_n % 8 == 0:
                unrolled(8)
            elif loop_n % 4 == 0:
                unrolled(4)
            elif loop_n % 2 == 0:
                unrolled(2)
            else:
                with rep_ctx(loop_n):
                    emit_body(0)

    nc.compile()
    return nc


def make_mask() -> np.ndarray:
    """[128, 512] 0/1 bf16 mask, tiled over (2 sub-tiles x 4 heads).
    Score tile: partition p = key (t0 - 64 + p), col i = query (t0 + i);
    in-band iff i+1 <= p <= i+64."""
    m = np.zeros((128, 64), dtype=np.float32)
    for i in range(64):
        m[i + 1:i + 65, i] = 1.0
    return np.tile(m, (1, 8)).astype(ml_dtypes.bfloat16)


def prep_inputs(x, Wq, Wk, Wv, Wo, last_k_init, last_v_init):
    """Shard + pre-transpose full inputs into 8 per-core bf16 input maps."""
    bf = ml_dtypes.bfloat16
    mask = make_mask()
    in_maps = []
    for core in range(NCORES):
        b, g = divmod(core, G)
        sl = slice(g * DG, (g + 1) * DG)
        lk = last_k_init[:, g * HPG:(g + 1) * HPG, :]   # [63, 4, 64]
        lv = last_v_init[:, g * HPG:(g + 1) * HPG, :]
        wqkv = np.concatenate([
            (Wq[sl, :] * (DH ** -0.5)).T, Wk[sl, :].T, Wv[sl, :].T,
        ], axis=1)  # [1024 din, 768]
        kc = np.zeros((64, HPG * 64), dtype=np.float32)
        vc = np.zeros((64, VROW), dtype=np.float32)
        for h in range(HPG):
            kc[:, h * 64 + 1: h * 64 + 64] = lk[:, h, :].T
            vc[1:64, h * VSLOT: h * VSLOT + DH] = lv[:, h, :]
        # pre-rearrange to [128, ...] sbuf layouts (one contiguous run per
        # partition per DMA): xT cols = n*4096 + k*512 + c;
        # w cols = k*512 + j (qk part), then 4096 + k*256 (v part)
        xr = x[b].T.reshape(8, 128, 2, 512).transpose(1, 2, 0, 3)
        wqk = wqkv[:, 0:512].reshape(8, 128, 512).transpose(1, 0, 2)
        wv2 = wqkv[:, 512:768].reshape(8, 128, 256).transpose(1, 0, 2)
        wr = np.concatenate([wqk.reshape(128, 4096),
                             wv2.reshape(128, 2048)], axis=1)
        wor = Wo[:, sl].T.reshape(2, 128, D).transpose(1, 0, 2)
        in_maps.append({
            "xT": np.ascontiguousarray(xr.reshape(128, 8192)).astype(bf),
            "wqkv": np.ascontiguousarray(wr).astype(bf),
            "woT": np.ascontiguousarray(wor.reshape(128, 2 * D)).astype(bf),
            "kc": kc.astype(bf),
            "vc": vc.astype(bf),
            "mask": mask,
        })
    return in_maps


_built = None


def kernel(x, Wq, Wk, Wv, Wo, last_k_init, last_v_init) -> np.ndarray:
    global _built
    x = np.asarray(x, dtype=np.float32)
    args = [np.asarray(a, dtype=np.float32)
            for a in (Wq, Wk, Wv, Wo, last_k_init, last_v_init)]
    in_maps = prep_inputs(x, *args)
    if _built is None:
        _built = build()
    r = bass_utils.run_bass_kernel_spmd(
        _built, in_maps, core_ids=list(range(NCORES)))
    out = np.zeros((B, L, D), dtype=np.float32)
    for core in range(NCORES):
        b = core // G
        out[b] += r.results[core]["y"].astype(np.float32)
    return out



# revision 2
# speedup vs baseline: 2.2087x; 2.2087x over previous
"""Banded multi-head attention (B=2, L=1024, D=1024, H=16, band W=64) on 8
Trainium2 NeuronCores.

Sharding: core = (batch b, head-group g), 2 batches x 4 head groups of 4
heads.  Each core computes q/k/v projections for its group, the banded
attention for its 4 heads, and a partial output projection through its
slice of Wo.  Host sums the 4 bf16 partial outputs per batch in f32.

v3 (vs the 139.8us gpsimd softmax design): ZERO Pool-engine instructions.
HW ablations showed the Q7 path (partition_all_reduce ~3us+, bf16
tensor ops ~12us each (!), SWDGE dma triggers) is many-fold slower than
CoreSim models it, and dominated the measured time:
- Softmax denominators on the PE: one all-ones [128, 128] stationary
  matmul per masked-e tile computes the key sums already broadcast to
  all 128 output partitions (written into the other, dead-after-exp
  score psum bank); one DVE reciprocal -> bf16 tile; one DVE 2x multiply
  normalizes e in place before attn@V.  The o copy is a plain copy.
- Band mask applied on the DVE (bf16 2x mode) instead of gpsimd.
- All DMA triggers on the HWDGE queues (x loads + vshifts + half the y
  stores on the otherwise idle SP queue, the rest on the Act queue).
- Engine balance: qk-projection copies split Act/DVE per head, v copies
  alternate Act/DVE, y copies split by half.
HW truths baked in (measured via loop-differential ablations):
- Every PE matmul costs ~60-90ns of issue overhead on top of its
  moving-row streaming, independent of stationary reuse, accumulation
  chaining, or psum bank choice.  328 matmuls/body -> ~26us of issue
  cost on top of the 34.1us streaming floor; total measured 64.5us/iter
  (prior gpsimd design: 139.8us).
- Quadrant matmuls (stationary/moving/out at partition base 64) wedge
  the device through this toolchain; everything here stays at base 0.
- x/y HBM traffic, DMA trigger counts, and the softmax cross-engine
  chain were each measured at <2us impact; do not contort for them.
The timing loop unrolls 16 bodies per For_i trip with double-buffered
x, cross-body prefetch, and cross-body PE primer chains (each body tail
emits the next body's first two projection chains).
"""
import numpy as np
import ml_dtypes

import concourse.bacc as bacc
import concourse.bass_isa as bass_isa
import concourse.mybir as mybir
import concourse.tile as tile
from concourse import bass_utils

B, L, D, H, W = 2, 1024, 1024, 16, 64
DH = D // H           # 64
G = 4                 # head groups
HPG = H // G          # 4 heads per group
DG = D // G           # 256 dims per group
NCORES = 8

VSLOT = DH            # 64 cols per (slot, head)
VROW = HPG * VSLOT    # 256 cols per key slot
NSLOT = 8             # 128-key slots

F32 = mybir.dt.float32
BF16 = mybir.dt.bfloat16
EXPF = mybir.ActivationFunctionType.Exp
MULT = None  # set lazily


def _pin_exp_table(arch: str):
    """Resolve Copy/Exp only to the natural_log_exp_and_others act-func
    set so exactly one table load is emitted (alternating per-function
    table swaps wedge the device)."""
    import concourse.hw_specs as hw_specs
    tables = hw_specs.get_activation_tables(arch)   # cached, mutable
    drop = {EXPF, mybir.ActivationFunctionType.Ln,
            mybir.ActivationFunctionType.Copy,
            mybir.ActivationFunctionType.Identity}
    assert "natural_log_exp_and_others" in tables
    for name, funcs in tables.items():
        if name != "natural_log_exp_and_others":
            funcs -= drop


def build(loop_n: int = 0):
    """Build + compile the per-core Bass program.  loop_n > 0 wraps the
    body in a device-side For_i executing it loop_n times (HW timing)."""
    from concourse.alu_op_type import AluOpType
    nc = bacc.Bacc("TRN2", target_bir_lowering=False, debug=False)
    _pin_exp_table(nc.m.arch)

    # Host pre-rearranges big tensors to [128, ...] so every DMA is one
    # contiguous run per partition (descriptor-count == 128; the DGE
    # trigger cost scales with descriptor count).
    xT = nc.dram_tensor("xT", [128, 2 * 8 * 512], BF16, kind="ExternalInput")
    wqkv = nc.dram_tensor("wqkv", [128, 8 * 768], BF16, kind="ExternalInput")
    woT = nc.dram_tensor("woT", [128, 2 * D], BF16, kind="ExternalInput")
    kcd = nc.dram_tensor("kc", [64, HPG * 64], BF16, kind="ExternalInput")
    vcd = nc.dram_tensor("vc", [64, VROW], BF16, kind="ExternalInput")
    maskd = nc.dram_tensor("mask", [128, 512], BF16, kind="ExternalInput")
    auxd = nc.dram_tensor("aux", [128, 136], BF16, kind="ExternalInput")
    y = nc.dram_tensor("y", [L, D], BF16, kind="ExternalOutput")

    with tile.TileContext(nc) as tc:
        with tc.tile_pool(name="res", bufs=1) as res, \
             tc.tile_pool(name="epool", bufs=4) as epool, \
             tc.tile_pool(name="rcpool", bufs=4) as rcpool, \
             tc.tile_pool(name="bcpool", bufs=4) as bcpool, \
             tc.tile_pool(name="ypool", bufs=3) as ypool, \
             tc.tile_pool(name="ps", bufs=8, space="PSUM") as psp:

            # ---- resident SBUF tensors ----------------------------------
            # xk col layout: n*4096 + k*512 + c  (n = token half, k = K
            # chunk, c = token within half) — matches the xT host layout.
            xkb = [res.tile([128, 8 * L], BF16, tag=f"xk{r}",
                             name=f"xk{r}") for r in range(2)]
            xkc = [xkb[0]]   # current rep's x buffer
            wsb = res.tile([128, 8 * 3 * DG], BF16, tag="w", name="w_sb")
            wo_sb = res.tile([128, 2 * D], BF16, tag="wo", name="wo_sb")
            qT = res.tile([64, HPG * L], BF16, tag="qT", name="qT")
            kT = res.tile([64, HPG * (64 + L)], BF16, tag="kT", name="kT")
            v_sb = res.tile([128, NSLOT * VROW], BF16, tag="v", name="v_sb")
            # v2 split into slot-pair tiles so attnV blocks only depend on
            # their own shift DMA (dependency tracking is per-tile)
            v2p = [res.tile([128, 2 * VROW], BF16, tag=f"v2_{i}",
                            name=f"v2_{i}") for i in range(4)]
            mask_sb = res.tile([128, 512], BF16, tag="mask", name="mask_sb")
            aux_sb = res.tile([128, 136], BF16, tag="aux", name="aux_sb")
            oT = [res.tile([128, L], BF16, tag=f"oT{m}", name=f"oT{m}")
                  for m in range(2)]

            KW = 64 + L  # kT cols per head

            def emit_qkproj(proj, m, n, c0=0, cw=512):
                """proj 0 = Q -> qT, 1 = K -> kT; m = head pair, n = token
                half, [c0, c0+cw) column window within the half.  psum
                [128 dims, cw tok] accumulated over 8 K-chunks, then
                per-head [64, cw] copies (Act + DVE)."""
                pt = psp.tile([128, 512], F32, tag="ps", name="pj")
                for k in range(8):
                    nc.tensor.matmul(
                        pt[:, 0:cw],
                        wsb[:, k * 512 + proj * 256 + m * 128:
                            k * 512 + proj * 256 + m * 128 + 128],
                        xkc[0][:, n * 4096 + k * 512 + c0:
                               n * 4096 + k * 512 + c0 + cw],
                        start=(k == 0), stop=(k == 7),
                    )
                for hh in range(2):
                    h = 2 * m + hh
                    if proj == 0:
                        dst = qT[:, h * L + n * 512 + c0:
                                 h * L + n * 512 + c0 + cw]
                    else:
                        dst = kT[:, h * KW + 64 + n * 512 + c0:
                                 h * KW + 64 + n * 512 + c0 + cw]
                    src = pt[hh * 64:(hh + 1) * 64, 0:cw]
                    if hh == 0:
                        nc.scalar.copy(dst, src)
                    else:
                        nc.vector.tensor_copy(dst, src)

            def emit_vproj(s):
                """Token slot s (128 tokens) -> v_sb B-slot s."""
                pv = psp.tile([128, 512], F32, tag="ps", name="pjv")
                n, c0 = divmod(s, 4)
                for k in range(8):
                    base = n * 4096 + k * 512 + c0 * 128
                    nc.tensor.matmul(
                        pv[:, 0:DG],
                        xkc[0][:, base: base + 128],
                        wsb[:, 4096 + k * 256: 4096 + k * 256 + 256],
                        start=(k == 0), stop=(k == 7),
                    )
                if s % 2 == 0:
                    nc.scalar.copy(v_sb[:, s * VROW:(s + 1) * VROW],
                                   pv[:, 0:DG])
                else:
                    nc.vector.tensor_copy(v_sb[:, s * VROW:(s + 1) * VROW],
                                          pv[:, 0:DG])

            eblk = {}

            def emit_attn_block(bi, phase):
                """256 queries [256*bi, 256*bi+256), all 4 heads.
                Tiles j = 4bi .. 4bi+3 (64 queries each); per 128q group one
                [128, 512] score psum (2 tiles x 4 heads x 64q).  phase
                'scores' emits scores+exp+mask; 'attnv' the rest."""
                if phase == "scores":
                    es, sts = [], []
                    for gg in range(2):
                        st = psp.tile([128, 512], F32, tag="ps", name="st")
                        for jj in range(2):
                            j = 4 * bi + 2 * gg + jj
                            for h in range(HPG):
                                nc.tensor.matmul(
                                    st[:, jj * 256 + h * 64:
                                       jj * 256 + h * 64 + 64],
                                    kT[:, h * KW + 64 * j:
                                       h * KW + 64 * j + 128],
                                    qT[:, h * L + 64 * j:
                                       h * L + 64 * j + 64],
                                    start=True, stop=True,
                                )
                        e = epool.tile([128, 512], BF16, tag="e", name="e")
                        nc.scalar.activation(e[:], st[:], EXPF)
                        nc.vector.tensor_mul(e[:], e[:], mask_sb[:])
                        es.append(e)
                        sts.append(st)
                    eblk[bi] = (es, sts)
                    return
                if phase == "recip":
                    es, sts = eblk[bi][0], eblk[bi][1]
                    for gg in range(2):
                        nc.tensor.matmul(
                            sts[1 - gg][:, 0:512],
                            aux_sb[:, 8:136], es[gg][:],
                            start=True, stop=True,
                        )
                    rbs = [rcpool.tile([128, 512], BF16, tag="rb",
                                       name="rb") for _ in range(2)]
                    eblk[bi] = (es, sts, rbs)
                    with nc.allow_low_precision(
                            reason="bf16 softmax reciprocals, 2e-2 budget"):
                        for gg in range(2):
                            nc.vector.reciprocal(rbs[gg][:],
                                                 sts[1 - gg][:, 0:512])
                    for gg in range(2):
                        nc.vector.tensor_mul(es[gg][:], es[gg][:],
                                             rbs[gg][:])
                    return
                es = eblk.pop(bi)[0]
                for m in range(2):
                    op = psp.tile([64, 512], F32, tag="ps", name="op")
                    for hh in range(2):
                        h = 2 * m + hh
                        for q4 in range(4):
                            j = 4 * bi + q4
                            slot = j // 2
                            if j % 2 == 0:
                                vt, so = v2p[slot // 2], slot % 2
                            else:
                                vt, so = v_sb, slot
                            nc.tensor.matmul(
                                op[0:64, hh * 256 + q4 * 64:
                                   hh * 256 + q4 * 64 + 64],
                                vt[:, so * VROW + h * VSLOT:
                                   so * VROW + h * VSLOT + VSLOT],
                                es[q4 // 2][:, (q4 % 2) * 256 + h * 64:
                                             (q4 % 2) * 256 + h * 64 + 64],
                                start=True, stop=True,
                            )
                    for hh in range(2):
                        dst = oT[m][hh * 64:(hh + 1) * 64,
                                    256 * bi: 256 * bi + 256]
                        if hh == 0:
                            nc.scalar.copy(
                                dst, op[0:64, hh * 256: hh * 256 + 256])
                        else:
                            nc.vector.tensor_copy(
                                dst, op[0:64, hh * 256: hh * 256 + 256])

            def emit_oproj(t):
                """Token tile t (128 tokens) -> y row block (bf16).  Each
                512-col half is DMA'd as soon as its copy lands."""
                ysb = ypool.tile([128, D], BF16, tag="y", name="ysb")
                for n2 in range(2):
                    yp = psp.tile([128, 512], F32, tag="ps", name="yp")
                    for m in range(2):
                        nc.tensor.matmul(
                            yp[:],
                            oT[m][:, t * 128:(t + 1) * 128],
                            wo_sb[:, m * D + n2 * 512: m * D + n2 * 512 + 512],
                            start=(m == 0), stop=(m == 1),
                        )
                    if n2 == 0:
                        nc.scalar.copy(ysb[:, 0:512], yp[:])
                    else:
                        nc.vector.tensor_copy(ysb[:, 512:1024], yp[:])
                    eng = nc.sync if (t + n2) % 2 == 0 else nc.scalar
                    eng.dma_start(
                        y.ap()[t * 128:(t + 1) * 128,
                               n2 * 512:(n2 + 1) * 512],
                        ysb[:, n2 * 512:(n2 + 1) * 512])

            def emit_vshift(a):
                """v2 A-slots {a, a+1} (keys [128a-64, 128a+192)) via
                partition-shifted sbuf->sbuf copies from B-slots."""
                vt = v2p[a // 2]
                nc.sync.dma_start(vt[64:128, :],
                                  v_sb[0:64, a * VROW:(a + 2) * VROW])
                lo = max(a, 1)   # A0 lower half is the const cache block
                nc.sync.dma_start(
                    vt[0:64, (lo - a) * VROW:2 * VROW],
                    v_sb[64:128, (lo - 1) * VROW:(a + 1) * VROW])

            # ---- constant inputs: outside the timing loop ----------------
            # Split so the first qk weight chunks land ASAP on the cold run.
            nc.sync.dma_start(wsb[:, 0:512], wqkv.ap()[:, 0:512])
            nc.sync.dma_start(wsb[:, 512:2048], wqkv.ap()[:, 512:2048])
            nc.sync.dma_start(wsb[:, 2048:4096], wqkv.ap()[:, 2048:4096])
            nc.sync.dma_start(wsb[:, 4096:6144], wqkv.ap()[:, 4096:6144])
            nc.sync.dma_start(
                kT[:].rearrange("p (h c) -> p h c", c=KW)[:, :, 0:64],
                kcd.ap().rearrange("p (h c) -> p h c", c=64))
            nc.sync.dma_start(v2p[0][0:64, 0:VROW], vcd.ap())
            nc.sync.dma_start(mask_sb[:], maskd.ap())
            nc.sync.dma_start(aux_sb[:], auxd.ap())
            nc.sync.dma_start(wo_sb[:], woT.ap())

            import contextlib

            def rep_ctx(trips):
                if trips:
                    return tc.For_i(0, trips, 1,
                                    hint_engines=(mybir.EngineType.PE,
                                                  mybir.EngineType.Activation,
                                                  mybir.EngineType.DVE,
                                                  mybir.EngineType.SP))
                return contextlib.nullcontext()

            def emit_xload(buf):
                xk = xkb[buf]
                nc.sync.dma_start(xk[:, 0:512], xT.ap()[:, 0:512])
                nc.sync.dma_start(xk[:, 512:2048], xT.ap()[:, 512:2048])
                nc.sync.dma_start(xk[:, 2048:4096], xT.ap()[:, 2048:4096])
                nc.sync.dma_start(xk[:, 4096:8192], xT.ap()[:, 4096:8192])

            def emit_body(rep, prefetch_buf=None, skip_first=False,
                          tail_next=None):
                # Double-buffered x.  prefetch_buf=None: load this body's
                # buffer at its head (cold / non-unrolled path).  Otherwise
                # the buffer was loaded by the previous body (or the
                # pre-loop load) and this body instead prefetches the NEXT
                # body's buffer mid-stream, so the following body starts
                # with x fully resident.
                xkc[0] = xkb[rep]
                if prefetch_buf is None:
                    emit_xload(rep)

                # ---- first half: tok 0-511 ------------------------------
                # the m=0 chains may have been emitted as the previous
                # body's tail fillers (cross-body PE fill)
                if not skip_first:
                    emit_qkproj(0, 0, 0)
                    emit_qkproj(1, 0, 0)
                emit_qkproj(0, 1, 0)
                emit_qkproj(1, 1, 0)
                emit_vproj(0)
                emit_vproj(1)
                emit_vshift(0)
                # scores run ahead of attnV; second-half projection chains
                # are the PE fillers that hide exp/mask and denominator
                # chain latencies
                emit_attn_block(0, phase="scores")
                emit_vproj(2)
                emit_attn_block(0, phase="recip")
                emit_vproj(3)
                emit_vshift(2)
                emit_attn_block(1, phase="scores")
                emit_attn_block(0, phase="attnv")
                emit_attn_block(1, phase="recip")
                emit_qkproj(0, 0, 1, 192, 320)
                emit_attn_block(1, phase="attnv")
                if prefetch_buf is not None:
                    emit_xload(prefetch_buf)
                emit_qkproj(1, 0, 1, 192, 320)
                # ---- second half: tok 512-1023 --------------------------
                # qk-n1 is emitted band-aligned: tok [704, 1024) for all
                # head pairs first, so sc-b3 (queries 768+, keys 704+) can
                # start while tok [512, 704) is still projecting; all
                # oprojs act as PE fillers that hide the attention chains;
                # b3 runs before b2 so the last oprojs wait on the earlier
                # denominator chain
                emit_qkproj(0, 1, 1, 192, 320)
                emit_qkproj(1, 1, 1, 192, 320)
                emit_attn_block(3, phase="scores")
                emit_qkproj(0, 0, 1, 0, 192)
                emit_qkproj(1, 0, 1, 0, 192)
                emit_qkproj(0, 1, 1, 0, 192)
                emit_qkproj(1, 1, 1, 0, 192)
                emit_attn_block(2, phase="scores")
                emit_attn_block(3, phase="recip")
                emit_vproj(5)
                emit_vproj(6)
                emit_vproj(7)
                emit_vshift(6)
                emit_vproj(4)
                emit_vshift(4)
                emit_attn_block(2, phase="recip")
                emit_oproj(0)
                emit_oproj(1)
                emit_attn_block(3, phase="attnv")
                emit_oproj(2)
                emit_attn_block(2, phase="attnv")
                emit_oproj(3)
                if tail_next is not None:
                    xkc[0] = xkb[tail_next]
                    emit_qkproj(0, 0, 0)
                emit_oproj(6)
                emit_oproj(7)
                if tail_next is not None:
                    emit_qkproj(1, 0, 0)
                    xkc[0] = xkb[rep]
                emit_oproj(4)
                emit_oproj(5)

            def unrolled(nb):
                emit_xload(0)
                xkc[0] = xkb[0]
                emit_qkproj(0, 0, 0)   # primer for the first body
                emit_qkproj(1, 0, 0)
                with rep_ctx(loop_n // nb):
                    for r in range(nb):
                        emit_body(r % 2, prefetch_buf=(r + 1) % 2,
                                  skip_first=True,
                                  tail_next=(r + 1) % 2)

            if loop_n == 0:
                emit_body(0)
            elif loop_n % 16 == 0:
                unrolled(16)
            elif loop_n % 8 == 0:
                unrolled(8)
            elif loop_n % 4 == 0:
                unrolled(4)
            elif loop_n % 2 == 0:
                unrolled(2)
            else:
                with rep_ctx(loop_n):
                    emit_body(0)

    nc.compile()
    return nc


def make_mask() -> np.ndarray:
    """[128, 512] 0/1 bf16 mask, tiled over (2 sub-tiles x 4 heads).
    Score tile: partition p = key (t0 - 64 + p), col i = query (t0 + i);
    in-band iff i+1 <= p <= i+64."""
    m = np.zeros((128, 64), dtype=np.float32)
    for i in range(64):
        m[i + 1:i + 65, i] = 1.0
    return np.tile(m, (1, 8)).astype(ml_dtypes.bfloat16)


def make_aux():
    a = np.zeros((128, 136), dtype=np.float32)
    a[:, 0] = 1.0
    a[:, 8:136] = 1.0
    return a.astype(ml_dtypes.bfloat16)


def prep_inputs(x, Wq, Wk, Wv, Wo, last_k_init, last_v_init):
    """Shard + pre-transpose full inputs into 8 per-core bf16 input maps."""
    bf = ml_dtypes.bfloat16
    mask = make_mask()
    aux = make_aux()
    in_maps = []
    for core in range(NCORES):
        b, g = divmod(core, G)
        sl = slice(g * DG, (g + 1) * DG)
        lk = last_k_init[:, g * HPG:(g + 1) * HPG, :]   # [63, 4, 64]
        lv = last_v_init[:, g * HPG:(g + 1) * HPG, :]
        wqkv = np.concatenate([
            (Wq[sl, :] * (DH ** -0.5)).T, Wk[sl, :].T, Wv[sl, :].T,
        ], axis=1)  # [1024 din, 768]
        kc = np.zeros((64, HPG * 64), dtype=np.float32)
        vc = np.zeros((64, VROW), dtype=np.float32)
        for h in range(HPG):
            kc[:, h * 64 + 1: h * 64 + 64] = lk[:, h, :].T
            vc[1:64, h * VSLOT: h * VSLOT + DH] = lv[:, h, :]
        # pre-rearrange to [128, ...] sbuf layouts (one contiguous run per
        # partition per DMA): xT cols = n*4096 + k*512 + c;
        # w cols = k*512 + j (qk part), then 4096 + k*256 (v part)
        xr = x[b].T.reshape(8, 128, 2, 512).transpose(1, 2, 0, 3)
        wqk = wqkv[:, 0:512].reshape(8, 128, 512).transpose(1, 0, 2)
        wv2 = wqkv[:, 512:768].reshape(8, 128, 256).transpose(1, 0, 2)
        wr = np.concatenate([wqk.reshape(128, 4096),
                             wv2.reshape(128, 2048)], axis=1)
        wor = Wo[:, sl].T.reshape(2, 128, D).transpose(1, 0, 2)
        in_maps.append({
            "xT": np.ascontiguousarray(xr.reshape(128, 8192)).astype(bf),
            "wqkv": np.ascontiguousarray(wr).astype(bf),
            "woT": np.ascontiguousarray(wor.reshape(128, 2 * D)).astype(bf),
            "kc": kc.astype(bf),
            "vc": vc.astype(bf),
            "mask": mask,
            "aux": aux,
        })
    return in_maps


_built = None


def kernel(x, Wq, Wk, Wv, Wo, last_k_init, last_v_init) -> np.ndarray:
    global _built
    x = np.asarray(x, dtype=np.float32)
    args = [np.asarray(a, dtype=np.float32)
            for a in (Wq, Wk, Wv, Wo, last_k_init, last_v_init)]
    in_maps = prep_inputs(x, *args)
    if _built is None:
        _built = build()
    r = bass_utils.run_bass_kernel_spmd(
        _built, in_maps, core_ids=list(range(NCORES)))
    out = np.zeros((B, L, D), dtype=np.float32)
    for core in range(NCORES):
        b = core // G
        out[b] += r.results[core]["y"].astype(np.float32)
    return out



# revision 3
# speedup vs baseline: 2.3118x; 1.0467x over previous
"""Banded multi-head attention (B=2, L=1024, D=1024, H=16, band W=64) on 8
Trainium2 NeuronCores.

Sharding: core = (batch b, head-group g), 2 batches x 4 head groups of 4
heads.  Each core computes q/k/v projections for its group, the banded
attention for its 4 heads, and a partial output projection through its
slice of Wo.  Host sums the 4 bf16 partial outputs per batch in f32.

v3 (vs the 139.8us gpsimd softmax design): ZERO Pool-engine instructions.
HW ablations showed the Q7 path (partition_all_reduce ~3us+, bf16
tensor ops ~12us each (!), SWDGE dma triggers) is many-fold slower than
CoreSim models it, and dominated the measured time:
- Softmax denominators on the PE: one all-ones [128, 128] stationary
  matmul per masked-e tile computes the key sums already broadcast to
  all 128 output partitions (written into the other, dead-after-exp
  score psum bank); one DVE reciprocal -> bf16 tile; one DVE 2x multiply
  normalizes e in place before attn@V.  The o copy is a plain copy.
- Band mask applied on the DVE (bf16 2x mode) instead of gpsimd.
- All DMA triggers on the HWDGE queues (x loads + vshifts + half the y
  stores on the otherwise idle SP queue, the rest on the Act queue).
- Engine balance: qk-projection copies split Act/DVE per head, v copies
  alternate Act/DVE, y copies split by half.
HW truths baked in (measured via loop-differential ablations):
- Every PE matmul costs ~60-90ns of issue overhead on top of its
  moving-row streaming, independent of stationary reuse, accumulation
  chaining, or psum bank choice.  328 matmuls/body -> ~26us of issue
  cost on top of the 34.1us streaming floor; total measured 64.5us/iter
  (prior gpsimd design: 139.8us).
- Quadrant matmuls (stationary/moving/out at partition base 64) wedge
  the device through this toolchain; everything here stays at base 0.
- x/y HBM traffic, DMA trigger counts, and the softmax cross-engine
  chain were each measured at <2us impact; do not contort for them.
The timing loop unrolls 16 bodies per For_i trip with double-buffered
x, cross-body prefetch, and cross-body PE primer chains (each body tail
emits the next body's first two projection chains).
"""
import numpy as np
import ml_dtypes

import concourse.bacc as bacc
import concourse.bass_isa as bass_isa
import concourse.mybir as mybir
import concourse.tile as tile
from concourse import bass_utils

B, L, D, H, W = 2, 1024, 1024, 16, 64
DH = D // H           # 64
G = 4                 # head groups
HPG = H // G          # 4 heads per group
DG = D // G           # 256 dims per group
NCORES = 8

VSLOT = DH            # 64 cols per (slot, head)
VROW = HPG * VSLOT    # 256 cols per key slot
NSLOT = 8             # 128-key slots

F32 = mybir.dt.float32
BF16 = mybir.dt.bfloat16
EXPF = mybir.ActivationFunctionType.Exp
MULT = None  # set lazily


def _pin_exp_table(arch: str):
    """Resolve Copy/Exp only to the natural_log_exp_and_others act-func
    set so exactly one table load is emitted (alternating per-function
    table swaps wedge the device)."""
    import concourse.hw_specs as hw_specs
    tables = hw_specs.get_activation_tables(arch)   # cached, mutable
    drop = {EXPF, mybir.ActivationFunctionType.Ln,
            mybir.ActivationFunctionType.Copy,
            mybir.ActivationFunctionType.Identity}
    assert "natural_log_exp_and_others" in tables
    for name, funcs in tables.items():
        if name != "natural_log_exp_and_others":
            funcs -= drop


def build(loop_n: int = 0):
    """Build + compile the per-core Bass program.  loop_n > 0 wraps the
    body in a device-side For_i executing it loop_n times (HW timing)."""
    from concourse.alu_op_type import AluOpType
    nc = bacc.Bacc("TRN2", target_bir_lowering=False, debug=False)
    _pin_exp_table(nc.m.arch)

    # Host pre-rearranges big tensors to [128, ...] so every DMA is one
    # contiguous run per partition (descriptor-count == 128; the DGE
    # trigger cost scales with descriptor count).
    xT = nc.dram_tensor("xT", [128, 2 * 8 * 512], BF16, kind="ExternalInput")
    wqkv = nc.dram_tensor("wqkv", [128, 8 * 768], BF16, kind="ExternalInput")
    woT = nc.dram_tensor("woT", [128, 2 * D], BF16, kind="ExternalInput")
    kcd = nc.dram_tensor("kc", [64, HPG * 64], BF16, kind="ExternalInput")
    vcd = nc.dram_tensor("vc", [64, VROW], BF16, kind="ExternalInput")
    maskd = nc.dram_tensor("mask", [128, 512], BF16, kind="ExternalInput")
    auxd = nc.dram_tensor("aux", [128, 136], BF16, kind="ExternalInput")
    y = nc.dram_tensor("y", [L, D], BF16, kind="ExternalOutput")

    with tile.TileContext(nc) as tc:
        with tc.tile_pool(name="res", bufs=1) as res, \
             tc.tile_pool(name="epool", bufs=4) as epool, \
             tc.tile_pool(name="rcpool", bufs=4) as rcpool, \
             tc.tile_pool(name="bcpool", bufs=4) as bcpool, \
             tc.tile_pool(name="ypool", bufs=3) as ypool, \
             tc.tile_pool(name="ps", bufs=8, space="PSUM") as psp:

            # ---- resident SBUF tensors ----------------------------------
            # xk col layout: n*4096 + k*512 + c  (n = token half, k = K
            # chunk, c = token within half) — matches the xT host layout.
            xkb = [res.tile([128, 8 * L], BF16, tag=f"xk{r}",
                             name=f"xk{r}") for r in range(2)]
            xkc = [xkb[0]]   # current rep's x buffer
            wsb = res.tile([128, 8 * 3 * DG], BF16, tag="w", name="w_sb")
            wo_sb = res.tile([128, 2 * D], BF16, tag="wo", name="wo_sb")
            qT = res.tile([64, HPG * L], BF16, tag="qT", name="qT")
            kT = res.tile([64, HPG * (64 + L)], BF16, tag="kT", name="kT")
            v_sb = res.tile([128, NSLOT * VROW], BF16, tag="v", name="v_sb")
            # v2 split into slot-pair tiles so attnV blocks only depend on
            # their own shift DMA (dependency tracking is per-tile)
            v2p = [res.tile([128, 2 * VROW], BF16, tag=f"v2_{i}",
                            name=f"v2_{i}") for i in range(4)]
            mask_sb = res.tile([128, 512], BF16, tag="mask", name="mask_sb")
            aux_sb = res.tile([128, 136], BF16, tag="aux", name="aux_sb")
            oT = [res.tile([128, L], BF16, tag=f"oT{m}", name=f"oT{m}")
                  for m in range(2)]

            KW = 64 + L  # kT cols per head

            def emit_qkproj(proj, m, n, c0=0, cw=512):
                """proj 0 = Q -> qT, 1 = K -> kT; m = head pair, n = token
                half, [c0, c0+cw) column window within the half.  psum
                [128 dims, cw tok] accumulated over 8 K-chunks, then
                per-head [64, cw] copies (Act + DVE)."""
                pt = psp.tile([128, 512], F32, tag="ps", name="pj")
                for k in range(8):
                    nc.tensor.matmul(
                        pt[:, 0:cw],
                        wsb[:, k * 512 + proj * 256 + m * 128:
                            k * 512 + proj * 256 + m * 128 + 128],
                        xkc[0][:, n * 4096 + k * 512 + c0:
                               n * 4096 + k * 512 + c0 + cw],
                        start=(k == 0), stop=(k == 7),
                    )
                for hh in range(2):
                    h = 2 * m + hh
                    if proj == 0:
                        dst = qT[:, h * L + n * 512 + c0:
                                 h * L + n * 512 + c0 + cw]
                    else:
                        dst = kT[:, h * KW + 64 + n * 512 + c0:
                                 h * KW + 64 + n * 512 + c0 + cw]
                    src = pt[hh * 64:(hh + 1) * 64, 0:cw]
                    if hh == 0:
                        nc.scalar.copy(dst, src)
                    else:
                        nc.vector.tensor_copy(dst, src)

            def emit_vproj(s):
                """Token slot s (128 tokens) -> v_sb B-slot s."""
                pv = psp.tile([128, 512], F32, tag="ps", name="pjv")
                n, c0 = divmod(s, 4)
                for k in range(8):
                    base = n * 4096 + k * 512 + c0 * 128
                    nc.tensor.matmul(
                        pv[:, 0:DG],
                        xkc[0][:, base: base + 128],
                        wsb[:, 4096 + k * 256: 4096 + k * 256 + 256],
                        start=(k == 0), stop=(k == 7),
                    )
                if s % 2 == 0:
                    nc.scalar.copy(v_sb[:, s * VROW:(s + 1) * VROW],
                                   pv[:, 0:DG])
                else:
                    nc.vector.tensor_copy(v_sb[:, s * VROW:(s + 1) * VROW],
                                          pv[:, 0:DG])

            eblk = {}

            def emit_attn_block(bi, phase):
                """256 queries [256*bi, 256*bi+256), all 4 heads.
                Tiles j = 4bi .. 4bi+3 (64 queries each); per 128q group one
                [128, 512] score psum (2 tiles x 4 heads x 64q).  phase
                'scores' emits scores+exp+mask; 'attnv' the rest."""
                if phase == "scores":
                    es, sts = [], []
                    for gg in range(2):
                        st = psp.tile([128, 512], F32, tag="ps", name="st")
                        for jj in range(2):
                            j = 4 * bi + 2 * gg + jj
                            for h in range(HPG):
                                nc.tensor.matmul(
                                    st[:, jj * 256 + h * 64:
                                       jj * 256 + h * 64 + 64],
                                    kT[:, h * KW + 64 * j:
                                       h * KW + 64 * j + 128],
                                    qT[:, h * L + 64 * j:
                                       h * L + 64 * j + 64],
                                    start=True, stop=True,
                                )
                        e = epool.tile([128, 512], BF16, tag="e", name="e")
                        nc.scalar.activation(e[:], st[:], EXPF)
                        nc.vector.tensor_mul(e[:], e[:], mask_sb[:])
                        es.append(e)
                        sts.append(st)
                    eblk[bi] = (es, sts)
                    return
                if phase == "recip":
                    es, sts = eblk[bi][0], eblk[bi][1]
                    for gg in range(2):
                        nc.tensor.matmul(
                            sts[1 - gg][:, 0:512],
                            aux_sb[:, 8:136], es[gg][:],
                            start=True, stop=True,
                        )
                    rbs = [rcpool.tile([128, 512], BF16, tag="rb",
                                       name="rb") for _ in range(2)]
                    eblk[bi] = (es, sts, rbs)
                    with nc.allow_low_precision(
                            reason="bf16 softmax reciprocals, 2e-2 budget"):
                        for gg in range(2):
                            nc.vector.reciprocal(rbs[gg][:],
                                                 sts[1 - gg][:, 0:512])
                    for gg in range(2):
                        nc.vector.tensor_mul(es[gg][:], es[gg][:],
                                             rbs[gg][:])
                    return
                es = eblk.pop(bi)[0]
                for m in range(2):
                    op = psp.tile([64, 512], F32, tag="ps", name="op")
                    for hh in range(2):
                        h = 2 * m + hh
                        for q4 in range(4):
                            j = 4 * bi + q4
                            slot = j // 2
                            if j % 2 == 0:
                                vt, so = v2p[slot // 2], slot % 2
                            else:
                                vt, so = v_sb, slot
                            nc.tensor.matmul(
                                op[0:64, hh * 256 + q4 * 64:
                                   hh * 256 + q4 * 64 + 64],
                                vt[:, so * VROW + h * VSLOT:
                                   so * VROW + h * VSLOT + VSLOT],
                                es[q4 // 2][:, (q4 % 2) * 256 + h * 64:
                                             (q4 % 2) * 256 + h * 64 + 64],
                                start=True, stop=True,
                            )
                    for hh in range(2):
                        dst = oT[m][hh * 64:(hh + 1) * 64,
                                    256 * bi: 256 * bi + 256]
                        if hh == 0:
                            nc.scalar.copy(
                                dst, op[0:64, hh * 256: hh * 256 + 256])
                        else:
                            nc.vector.tensor_copy(
                                dst, op[0:64, hh * 256: hh * 256 + 256])

            def emit_oproj(t):
                """Token tile t (128 tokens) -> y row block (bf16).  Each
                512-col half is DMA'd as soon as its copy lands."""
                ysb = ypool.tile([128, D], BF16, tag="y", name="ysb")
                for n2 in range(2):
                    yp = psp.tile([128, 512], F32, tag="ps", name="yp")
                    for m in range(2):
                        nc.tensor.matmul(
                            yp[:],
                            oT[m][:, t * 128:(t + 1) * 128],
                            wo_sb[:, m * D + n2 * 512: m * D + n2 * 512 + 512],
                            start=(m == 0), stop=(m == 1),
                        )
                    if n2 == 0:
                        nc.scalar.copy(ysb[:, 0:512], yp[:])
                    else:
                        nc.vector.tensor_copy(ysb[:, 512:1024], yp[:])
                    eng = nc.sync if (t + n2) % 2 == 0 else nc.scalar
                    eng.dma_start(
                        y.ap()[t * 128:(t + 1) * 128,
                               n2 * 512:(n2 + 1) * 512],
                        ysb[:, n2 * 512:(n2 + 1) * 512])

            def emit_vshift(a):
                """v2 A-slots {a, a+1} (keys [128a-64, 128a+192)) via
                partition-shifted sbuf->sbuf copies from B-slots."""
                vt = v2p[a // 2]
                nc.sync.dma_start(vt[64:128, :],
                                  v_sb[0:64, a * VROW:(a + 2) * VROW])
                lo = max(a, 1)   # A0 lower half is the const cache block
                nc.sync.dma_start(
                    vt[0:64, (lo - a) * VROW:2 * VROW],
                    v_sb[64:128, (lo - 1) * VROW:(a + 1) * VROW])

            # ---- constant inputs: outside the timing loop ----------------
            # Split so the first qk weight chunks land ASAP on the cold run.
            nc.sync.dma_start(wsb[:, 0:512], wqkv.ap()[:, 0:512])
            nc.sync.dma_start(wsb[:, 512:2048], wqkv.ap()[:, 512:2048])
            nc.sync.dma_start(wsb[:, 2048:4096], wqkv.ap()[:, 2048:4096])
            nc.sync.dma_start(wsb[:, 4096:6144], wqkv.ap()[:, 4096:6144])
            nc.sync.dma_start(
                kT[:].rearrange("p (h c) -> p h c", c=KW)[:, :, 0:64],
                kcd.ap().rearrange("p (h c) -> p h c", c=64))
            nc.sync.dma_start(v2p[0][0:64, 0:VROW], vcd.ap())
            nc.sync.dma_start(mask_sb[:], maskd.ap())
            nc.sync.dma_start(aux_sb[:], auxd.ap())
            nc.sync.dma_start(wo_sb[:], woT.ap())

            import contextlib

            def rep_ctx(trips):
                if trips:
                    return tc.For_i(0, trips, 1,
                                    hint_engines=(mybir.EngineType.PE,
                                                  mybir.EngineType.Activation,
                                                  mybir.EngineType.DVE,
                                                  mybir.EngineType.SP))
                return contextlib.nullcontext()

            def emit_xload(buf):
                xk = xkb[buf]
                nc.sync.dma_start(xk[:, 0:512], xT.ap()[:, 0:512])
                nc.sync.dma_start(xk[:, 512:2048], xT.ap()[:, 512:2048])
                nc.sync.dma_start(xk[:, 2048:4096], xT.ap()[:, 2048:4096])
                nc.sync.dma_start(xk[:, 4096:8192], xT.ap()[:, 4096:8192])

            def emit_body(rep, prefetch_buf=None, skip_first=False,
                          tail_next=None):
                # Double-buffered x.  prefetch_buf=None: load this body's
                # buffer at its head (cold / non-unrolled path).  Otherwise
                # the buffer was loaded by the previous body (or the
                # pre-loop load) and this body instead prefetches the NEXT
                # body's buffer mid-stream, so the following body starts
                # with x fully resident.
                xkc[0] = xkb[rep]
                if prefetch_buf is None:
                    emit_xload(rep)

                # ---- first half: tok 0-511 ------------------------------
                # the m=0 chains may have been emitted as the previous
                # body's tail fillers (cross-body PE fill)
                if not skip_first:
                    emit_qkproj(0, 0, 0)
                    emit_qkproj(1, 0, 0)
                emit_qkproj(0, 1, 0)
                emit_qkproj(1, 1, 0)
                emit_vproj(0)
                emit_vproj(1)
                emit_vshift(0)
                # scores run ahead of attnV; second-half projection chains
                # are the PE fillers that hide exp/mask and denominator
                # chain latencies
                emit_attn_block(0, phase="scores")
                emit_vproj(2)
                emit_attn_block(0, phase="recip")
                emit_vproj(3)
                emit_vshift(2)
                emit_attn_block(1, phase="scores")
                emit_attn_block(0, phase="attnv")
                emit_attn_block(1, phase="recip")
                emit_qkproj(0, 0, 1, 192, 320)
                emit_attn_block(1, phase="attnv")
                if prefetch_buf is not None:
                    emit_xload(prefetch_buf)
                emit_qkproj(1, 0, 1, 192, 320)
                # ---- second half: tok 512-1023 --------------------------
                # qk-n1 is emitted band-aligned: tok [704, 1024) for all
                # head pairs first, so sc-b3 (queries 768+, keys 704+) can
                # start while tok [512, 704) is still projecting; all
                # oprojs act as PE fillers that hide the attention chains;
                # b3 runs before b2 so the last oprojs wait on the earlier
                # denominator chain
                emit_qkproj(0, 1, 1, 192, 320)
                emit_qkproj(1, 1, 1, 192, 320)
                emit_attn_block(3, phase="scores")
                emit_qkproj(0, 0, 1, 0, 192)
                emit_qkproj(1, 0, 1, 0, 192)
                emit_qkproj(0, 1, 1, 0, 192)
                emit_qkproj(1, 1, 1, 0, 192)
                emit_attn_block(2, phase="scores")
                emit_attn_block(3, phase="recip")
                emit_vproj(5)
                emit_vproj(6)
                emit_vproj(7)
                emit_vshift(6)
                emit_vproj(4)
                emit_vshift(4)
                emit_attn_block(2, phase="recip")
                emit_oproj(0)
                emit_oproj(1)
                emit_attn_block(3, phase="attnv")
                emit_oproj(2)
                emit_attn_block(2, phase="attnv")
                emit_oproj(3)
                if tail_next is not None:
                    xkc[0] = xkb[tail_next]
                    emit_qkproj(0, 0, 0)
                emit_oproj(6)
                emit_oproj(7)
                if tail_next is not None:
                    emit_qkproj(1, 0, 0)
                    xkc[0] = xkb[rep]
                emit_oproj(4)
                emit_oproj(5)

            def unrolled(nb):
                emit_xload(0)
                xkc[0] = xkb[0]
                emit_qkproj(0, 0, 0)   # primer for the first body
                emit_qkproj(1, 0, 0)
                with rep_ctx(loop_n // nb):
                    for r in range(nb):
                        emit_body(r % 2, prefetch_buf=(r + 1) % 2,
                                  skip_first=True,
                                  tail_next=(r + 1) % 2)

            if loop_n == 0:
                emit_body(0)
            elif loop_n % 16 == 0:
                unrolled(16)
            elif loop_n % 8 == 0:
                unrolled(8)
            elif loop_n % 4 == 0:
                unrolled(4)
            elif loop_n % 2 == 0:
                unrolled(2)
            else:
                with rep_ctx(loop_n):
                    emit_body(0)

    nc.compile()
    return nc


def make_mask() -> np.ndarray:
    """[128, 512] 0/1 bf16 mask, tiled over (2 sub-tiles x 4 heads).
    Score tile: partition p = key (t0 - 64 + p), col i = query (t0 + i);
    in-band iff i+1 <= p <= i+64."""
    m = np.zeros((128, 64), dtype=np.float32)
    for i in range(64):
        m[i + 1:i + 65, i] = 1.0
    return np.tile(m, (1, 8)).astype(ml_dtypes.bfloat16)


def make_aux():
    a = np.zeros((128, 136), dtype=np.float32)
    a[:, 0] = 1.0
    a[:, 8:136] = 1.0
    return a.astype(ml_dtypes.bfloat16)


def prep_inputs(x, Wq, Wk, Wv, Wo, last_k_init, last_v_init):
    """Shard + pre-transpose full inputs into 8 per-core bf16 input maps."""
    bf = ml_dtypes.bfloat16
    mask = make_mask()
    aux = make_aux()
    in_maps = []
    for core in range(NCORES):
        b, g = divmod(core, G)
        sl = slice(g * DG, (g + 1) * DG)
        lk = last_k_init[:, g * HPG:(g + 1) * HPG, :]   # [63, 4, 64]
        lv = last_v_init[:, g * HPG:(g + 1) * HPG, :]
        wqkv = np.concatenate([
            (Wq[sl, :] * (DH ** -0.5)).T, Wk[sl, :].T, Wv[sl, :].T,
        ], axis=1)  # [1024 din, 768]
        kc = np.zeros((64, HPG * 64), dtype=np.float32)
        vc = np.zeros((64, VROW), dtype=np.float32)
        for h in range(HPG):
            kc[:, h * 64 + 1: h * 64 + 64] = lk[:, h, :].T
            vc[1:64, h * VSLOT: h * VSLOT + DH] = lv[:, h, :]
        # pre-rearrange to [128, ...] sbuf layouts (one contiguous run per
        # partition per DMA): xT cols = n*4096 + k*512 + c;
        # w cols = k*512 + j (qk part), then 4096 + k*256 (v part)
        xr = x[b].T.reshape(8, 128, 2, 512).transpose(1, 2, 0, 3)
        wqk = wqkv[:, 0:512].reshape(8, 128, 512).transpose(1, 0, 2)
        wv2 = wqkv[:, 512:768].reshape(8, 128, 256).transpose(1, 0, 2)
        wr = np.concatenate([wqk.reshape(128, 4096),
                             wv2.reshape(128, 2048)], axis=1)
        wor = Wo[:, sl].T.reshape(2, 128, D).transpose(1, 0, 2)
        in_maps.append({
            "xT": np.ascontiguousarray(xr.reshape(128, 8192)).astype(bf),
            "wqkv": np.ascontiguousarray(wr).astype(bf),
            "woT": np.ascontiguousarray(wor.reshape(128, 2 * D)).astype(bf),
            "kc": kc.astype(bf),
            "vc": vc.astype(bf),
            "mask": mask,
            "aux": aux,
        })
    return in_maps


_built = None


def kernel(x, Wq, Wk, Wv, Wo, last_k_init, last_v_init) -> np.ndarray:
    global _built
    x = np.asarray(x, dtype=np.float32)
    args = [np.asarray(a, dtype=np.float32)
            for a in (Wq, Wk, Wv, Wo, last_k_init, last_v_init)]
    in_maps = prep_inputs(x, *args)
    if _built is None:
        _built = build()
    try:
        r = bass_utils.run_bass_kernel_spmd(
            _built, in_maps, core_ids=list(range(NCORES)))
    except Exception:
        # transient device wedge (e.g. NRT_EXEC_UNIT_UNRECOVERABLE) —
        # one retry is usually enough per the TRN2 runbook
        import time as _time
        _time.sleep(5)
        r = bass_utils.run_bass_kernel_spmd(
            _built, in_maps, core_ids=list(range(NCORES)))
    out = np.zeros((B, L, D), dtype=np.float32)
    for core in range(NCORES):
        b = core // G
        out[b] += r.results[core]["y"].astype(np.float32)
    return out

